# revision 1
# baseline (speedup 1.0000x reference)
"""AGNN (2-layer propagation) Trainium2 Bass kernel, 8-core SPMD — V2.

Architecture (dst-node-range sharding, 12500 nodes/core, LP=12544):
  - phase A: h0 = relu(x_local @ W1 + b1) via PE (node-major output).
  - per prop: L2-normalize, pack records (hn||norm) 4/256B-window, AllGather
    table to all cores; per-edge phase gathers source windows via gpsimd
    dma_gather into a flat ELL column stream (dst-row-major, degree-sorted
    rank permutation, equal-K tile runs), then computes the segment softmax
    and weighted sums with CONSOLIDATED per-chunk/per-run DVE ops (4D
    broadcast/strided APs) instead of per-tile ops.
  - self-loops are folded analytically (exact, via on-chip self-cos), ELL
    row pads are corrected by a precomputed pad count; softmax is shift-free
    (logits = beta*cos bounded).
  - classifier uses 16 DVE rank-1 updates (no PE transposes), log_softmax.
"""
import sys
import types

sys.path.insert(0, "/opt/trn_rl_repo")

import numpy as np

try:  # optional NTFF profiling hook (enabled by test.py via TRACE)
    import antenv
    if "antenv.axon_hooks" not in sys.modules:
        _hook = [None]
        _m = types.ModuleType("antenv.axon_hooks")
        _m.set_axon_ntff_profile_hook = lambda h: _hook.__setitem__(0, h)
        _m.get_axon_ntff_profile_hook = lambda: _hook[0]
        sys.modules["antenv.axon_hooks"] = _m
        antenv.axon_hooks = _m
        try:
            from trn_agent_boot.trn_boot import _ntff_profile_via_ctypes
            _m.set_axon_ntff_profile_hook(
                _ntff_profile_via_ctypes("/opt/axon/libaxon_pjrt.so"))
        except Exception:
            pass
except Exception:
    pass

import concourse.bass as bass  # noqa: F401
import concourse.mybir as mybir
import concourse.tile as tile
from concourse import bacc
from concourse import library_config
from concourse.bass_utils import run_bass_kernel_spmd
from concourse.masks import make_identity

F32 = mybir.dt.float32
BF16 = mybir.dt.bfloat16
U8 = mybir.dt.uint8
I16 = mybir.dt.int16
AF = mybir.ActivationFunctionType
OP = mybir.AluOpType
AX = mybir.AxisListType

NC_CORES = 8
N = 100000
F_IN = 1433
H = 16
C = 7
EPS = 1e-12

L = N // NC_CORES            # 12500 local nodes per core
LP = 12544                   # padded (98 tiles of 128)
NT = LP // 128               # 98 tiles
KP = 1536                    # padded contraction dim (12 x 128)
NW = NC_CORES * LP // 4      # 25088 real windows
DUMMY_W = NW                 # zero window
NTAB = NW + 4

NIDX_CALL = 1024             # dma_gather idxs per call (tested safe)
CC = NIDX_CALL // 128        # gather-call columns (16)
COL_W = NIDX_CALL // 16      # wrapped idx columns per call (128)
CHUNK_COLS = 128             # max ELL columns per G4 chunk

TRACE = [False]
LAST_EXEC_NS = [None]


def _bc(ap, shape):
    try:
        return ap.broadcast_to(shape)
    except Exception:
        return ap.to_broadcast(shape)


def _plan(Kprof):
    """Chunk the tile sequence into gather chunks (<= CHUNK_COLS columns,
    padded to CC multiples) of equal-K runs."""
    chunks = []
    t = 0
    col0 = 0
    while t < NT:
        tiles = []
        cols = 0
        while t < NT and cols + int(Kprof[t]) <= CHUNK_COLS:
            tiles.append(t)
            cols += int(Kprof[t])
            t += 1
        assert tiles, f"tile K {Kprof[t]} exceeds CHUNK_COLS"
        # equal-K runs within the chunk
        runs = []
        i = 0
        off = 0
        while i < len(tiles):
            K = int(Kprof[tiles[i]])
            j = i
            while j < len(tiles) and int(Kprof[tiles[j]]) == K:
                j += 1
            T = j - i
            if K > 0:
                runs.append((tiles[i], T, K, off))
            off += T * K
            i = j
        cols_pad = -(-cols // CC) * CC
        chunks.append(dict(tiles=tiles, runs=runs, cols=cols,
                           cols_pad=cols_pad, ncalls=cols_pad // CC,
                           col0=col0))
        col0 += cols_pad
    return chunks, col0


def _host_prep(x, edge_index, W1, b1, beta, W2, b2):
    src_all = edge_index[0].astype(np.int64)
    dst_all = edge_index[1].astype(np.int64)
    deg = np.bincount(dst_all, minlength=N)  # real edges only; loops folded

    perms = []
    deg_sorted = np.zeros((NC_CORES, LP), dtype=np.int64)
    for c in range(NC_CORES):
        dl = deg[c * L:(c + 1) * L]
        order = np.argsort(-dl, kind="stable")
        perms.append(order)
        deg_sorted[c, :L] = dl[order]

    Kprof = deg_sorted[:, ::128].max(axis=0).astype(np.int64)  # [NT]
    chunks, CTOT = _plan(Kprof)
    ncalls = CTOT // CC

    # tile -> (chunk col0 + in-chunk offset)
    tile_col = np.zeros(NT, dtype=np.int64)
    for ch in chunks:
        off = 0
        for t in ch["tiles"]:
            tile_col[t] = ch["col0"] + off
            off += int(Kprof[t])

    rank_of = np.empty(N, dtype=np.int64)
    for c in range(NC_CORES):
        rank_of[c * L + perms[c]] = np.arange(L)
    src_rank = rank_of[src_all]
    src_gp = (src_all // L) * LP + (src_rank % 128) * NT + (src_rank // 128)
    dstc = dst_all // L
    dst_rank = rank_of[dst_all]

    idx_streams = np.empty((NC_CORES, 128, ncalls * COL_W), dtype=np.int16)
    msks = np.zeros((NC_CORES, 128, 3 * CTOT), dtype=np.uint8)
    negpad = np.zeros((NC_CORES, 128, NT), dtype=np.float32)

    for c in range(NC_CORES):
        sel = dstc == c
        dr = dst_rank[sel]
        gp = src_gp[sel]
        o = np.argsort(dr, kind="stable")
        dr = dr[o]
        gp = gp[o]
        starts = np.searchsorted(dr, np.arange(LP))
        pos = np.arange(len(dr)) - starts[dr]
        tt = dr // 128
        pp = dr % 128
        col = tile_col[tt] + pos
        iw = np.full((128, CTOT), DUMMY_W, dtype=np.int64)
        sub = np.zeros((128, CTOT), dtype=np.int64)
        iw[pp, col] = gp >> 2
        sub[pp, col] = gp & 3
        real = np.zeros((128, CTOT), dtype=bool)
        real[pp, col] = True
        for j in (1, 2, 3):
            msks[c, :, (j - 1) * CTOT:j * CTOT] = (sub == j) & real

        lin = iw.T.reshape(ncalls, NIDX_CALL)  # col-major per call
        wr = lin.reshape(ncalls, COL_W, 16).transpose(0, 2, 1)
        flat = wr.transpose(1, 0, 2).reshape(16, ncalls * COL_W)
        for rep in range(8):
            idx_streams[c, rep * 16:(rep + 1) * 16, :] = flat

        kmat = np.repeat(Kprof[None, :], 128, axis=0)  # [128, NT]
        dmat = deg_sorted[c].reshape(NT, 128).T        # [p, t]
        negpad[c] = -(kmat - dmat).astype(np.float32)

    W1p = np.zeros((KP, H), dtype=np.float32)
    W1p[:F_IN] = W1
    in_maps = []
    for c in range(NC_CORES):
        xt = np.zeros((KP, LP), dtype=np.float32)
        xt[:F_IN, :L] = x[c * L + perms[c]].T
        in_maps.append({
            "xt": xt.astype(np.dtype("bfloat16")),
            "w1": W1p.astype(np.dtype("bfloat16")),
            "b1": b1.reshape(H, 1).astype(np.float32),
            "w2r": np.tile(W2.reshape(1, H * C), (128, 1)).astype(np.float32),
            "b2r": np.tile(b2.reshape(1, C), (128, 1)).astype(np.float32),
            "betar": np.full((128, 1), float(beta[0]), dtype=np.float32),
            "idxs": idx_streams[c],
            "msk": msks[c],
            "negpad": negpad[c],
        })
    meta = dict(chunks=chunks, CTOT=CTOT, ncalls=ncalls, Kprof=Kprof,
                perms=perms)
    return in_maps, meta


def _build_program(meta):
    chunks = meta["chunks"]
    CTOT = meta["CTOT"]
    ncalls_total = meta["ncalls"]

    nc = bacc.Bacc("TRN2", target_bir_lowering=False, debug=False,
                   num_devices=NC_CORES, num_swdge_queues=4)

    xt_d = nc.dram_tensor("xt", [KP, LP], BF16, kind="ExternalInput")
    w1_d = nc.dram_tensor("w1", [KP, H], BF16, kind="ExternalInput")
    b1_d = nc.dram_tensor("b1", [H, 1], F32, kind="ExternalInput")
    w2r_d = nc.dram_tensor("w2r", [128, H * C], F32, kind="ExternalInput")
    b2r_d = nc.dram_tensor("b2r", [128, C], F32, kind="ExternalInput")
    betar_d = nc.dram_tensor("betar", [128, 1], F32, kind="ExternalInput")
    idxs_d = nc.dram_tensor("idxs", [128, ncalls_total * COL_W], I16,
                            kind="ExternalInput")
    msk_d = nc.dram_tensor("msk", [128, 3 * CTOT], U8, kind="ExternalInput")
    negpad_d = nc.dram_tensor("negpad", [128, NT], F32, kind="ExternalInput")
    out_d = nc.dram_tensor("out", [LP, C], F32, kind="ExternalOutput")

    rec_d = [nc.dram_tensor(f"rec{i}", [128, NT, 32], BF16) for i in range(2)]
    tab_d = [nc.dram_tensor(f"tab{i}", [NTAB, 128], BF16, addr_space="Shared")
             for i in range(2)]

    qctr = [0]

    def nextq():
        q = qctr[0] % 4
        qctr[0] += 1
        return q

    with tile.TileContext(nc) as tc:
        with tc.tile_pool(name="const", bufs=1) as cst, \
             tc.tile_pool(name="state", bufs=1) as st, \
             tc.tile_pool(name="work", bufs=2) as wk, \
             tc.tile_pool(name="once", bufs=1) as one, \
             tc.tile_pool(name="gath", bufs=2) as gp:

            nc.gpsimd.load_library(library_config.mlp)

            w1sb = cst.tile([128, 12, H], BF16)
            for kt in range(12):
                nc.sync.dma_start(out=w1sb[:, kt, :],
                                  in_=w1_d[kt * 128:(kt + 1) * 128, :])
            b1sb = cst.tile([H, 1], F32)
            nc.sync.dma_start(out=b1sb[:], in_=b1_d[:])
            w2rsb = cst.tile([128, H * C], F32)
            nc.sync.dma_start(out=w2rsb[:], in_=w2r_d[:])
            b2rsb = cst.tile([128, C], F32)
            nc.sync.dma_start(out=b2rsb[:], in_=b2r_d[:])
            betasb = cst.tile([128, 1], F32)
            nc.sync.dma_start(out=betasb[:], in_=betar_d[:])
            epssb = cst.tile([128, 1], F32)
            nc.vector.memset(epssb[:], EPS)
            msksb = cst.tile([128, 3 * CTOT], U8)
            nc.sync.dma_start(out=msksb[:], in_=msk_d[:])
            negpadsb = cst.tile([128, NT], F32)
            nc.sync.dma_start(out=negpadsb[:], in_=negpad_d[:])
            ident128 = cst.tile([128, 128], F32)
            make_identity(nc, ident128[:])
            zer = cst.tile([1, 128], BF16)
            nc.vector.memset(zer[:], 0)
            for i in range(2):
                nc.sync.dma_start(out=tab_d[i][NW:NW + 1, :], in_=zer[:])

            # ------------- phase A: h0 = relu(x W1 + b1), node-major -------
            h0nm = st.tile([128, NT, H], F32)
            with tc.tile_pool(name="psA", bufs=1, space="PSUM") as psA, \
                 tc.tile_pool(name="psTa", bufs=4, space="PSUM") as psTa, \
                 tc.tile_pool(name="wkA", bufs=2) as wkA:
                CH = 1024
                for coff in range(0, LP, CH):
                    csz = min(CH, LP - coff)
                    ps = psA.tile([H, CH], F32, tag="psa")
                    for kt in range(12):
                        kr = min(128, F_IN - kt * 128)
                        xtile = wkA.tile([128, CH], BF16, tag="xt")
                        nc.sync.dma_start(
                            out=xtile[:kr, :csz],
                            in_=xt_d[kt * 128:kt * 128 + kr, coff:coff + csz])
                        for m in range(0, csz, 512):
                            mw = min(512, csz - m)
                            nc.tensor.matmul(ps[:, m:m + mw],
                                             lhsT=w1sb[:kr, kt, :],
                                             rhs=xtile[:kr, m:m + mw],
                                             start=(kt == 0), stop=(kt == 11))
                    hfm = wkA.tile([H, CH], F32, tag="hfm")
                    nc.scalar.activation(hfm[:, :csz], ps[:, :csz], AF.Relu,
                                         bias=b1sb[:])
                    for i in range(csz // 128):
                        tg = (coff + i * 128) // 128
                        pt = psTa.tile([128, H], F32, tag="pst")
                        nc.tensor.transpose(
                            out=pt[:], in_=hfm[:, i * 128:(i + 1) * 128],
                            identity=ident128[:H, :H])
                        nc.scalar.activation(h0nm[:, tg, :], pt[:], AF.Copy)

            hnbf = st.tile([128, NT, H], BF16)
            h1nm = st.tile([128, NT, H], F32)
            sgrp = st.tile([128, NT], F32)
            selfee = st.tile([128, NT], F32)

            def normalize_and_share(phase, use_beta):
                hh = one.tile([128, NT * H], F32, tag="hh")
                nc.scalar.activation(
                    hh[:], h0nm[:].rearrange("p t h -> p (t h)"), AF.Square)
                ss = one.tile([128, NT], F32, tag="ss")
                nc.vector.tensor_reduce(
                    ss[:], hh[:].rearrange("p (t h) -> p t h", h=H),
                    axis=AX.X, op=OP.add)
                sq = one.tile([128, NT], F32, tag="sq")
                nc.scalar.activation(sq[:], ss[:], AF.Sqrt, bias=epssb[:])
                rr = one.tile([128, NT], F32, tag="rr")
                nc.vector.reciprocal(rr[:], sq[:])
                # self-cos = ss/(ss+eps); exact self-loop term
                sc = one.tile([128, NT], F32, tag="sc")
                nc.vector.tensor_tensor(out=sc[:], in0=ss[:], in1=rr[:],
                                        op=OP.mult)
                nc.vector.tensor_tensor(out=sc[:], in0=sc[:], in1=rr[:],
                                        op=OP.mult)
                nc.scalar.activation(selfee[:], sc[:], AF.Exp,
                                     scale=(betasb[:] if use_beta else 1.0))
                nc.vector.tensor_tensor(
                    out=hnbf[:], in0=h0nm[:],
                    in1=_bc(rr[:].unsqueeze(2), [128, NT, H]),
                    op=OP.mult)
                rec = one.tile([128, NT, 32], BF16, tag="rec")
                nc.vector.tensor_copy(out=rec[:, :, 0:H], in_=hnbf[:])
                nc.vector.tensor_copy(out=rec[:, :, H:H + 1],
                                      in_=sq[:].unsqueeze(2))
                nc.sync.dma_start(out=rec_d[phase][:], in_=rec[:])
                nc.gpsimd.collective_compute(
                    "AllGather", OP.bypass,
                    replica_groups=[list(range(NC_CORES))],
                    ins=[rec_d[phase][:]],
                    outs=[tab_d[phase][0:NW, :]],
                )

            def prop(phase, use_beta):
                nc.vector.memset(h1nm[:], 0)
                nc.vector.memset(sgrp[:], 0)
                for ch in chunks:
                    cols = ch["cols"]
                    ncalls = ch["ncalls"]
                    cb = ch["col0"] // CC
                    idxsb = wk.tile([128, (CHUNK_COLS // CC) * COL_W], I16,
                                    tag="idx")
                    nc.sync.dma_start(
                        out=idxsb[:, :ncalls * COL_W],
                        in_=idxs_d[:, cb * COL_W:(cb + ncalls) * COL_W])
                    G4 = gp.tile([128, CHUNK_COLS, 128], BF16, tag="g4")
                    for i in range(ncalls):
                        nc.gpsimd.dma_gather(
                            out_ap=G4[:, i * CC:(i + 1) * CC, :],
                            in_ap=tab_d[phase][:],
                            idxs_ap=idxsb[:, i * COL_W:(i + 1) * COL_W],
                            num_idxs=NIDX_CALL,
                            num_idxs_reg=NIDX_CALL,
                            elem_size=128,
                            queue_num=nextq(),
                        )
                    Gs = wk.tile([128, CHUNK_COLS, 18], BF16, tag="gs")
                    nc.scalar.activation(Gs[:, :cols, :], G4[:, :cols, 0:18],
                                         AF.Copy)
                    mc0 = ch["col0"]
                    for j in (1, 2, 3):
                        mj = msksb[:, (j - 1) * CTOT + mc0:
                                   (j - 1) * CTOT + mc0 + cols]
                        nc.vector.copy_predicated(
                            out=Gs[:, :cols, :],
                            mask=_bc(mj.unsqueeze(2), [128, cols, 18]),
                            data=G4[:, :cols, 32 * j:32 * j + 18])
                    prod = wk.tile([128, CHUNK_COLS, H], BF16, tag="prod")
                    for (t0, T, K, off) in ch["runs"]:
                        nc.vector.tensor_tensor(
                            out=prod[:, off:off + T * K, :].rearrange(
                                "p (t k) h -> p t k h", k=K),
                            in0=Gs[:, off:off + T * K, 0:H].rearrange(
                                "p (t k) h -> p t k h", k=K),
                            in1=_bc(hnbf[:, t0:t0 + T, :].unsqueeze(2),
                                    [128, T, K, H]),
                            op=OP.mult)
                    cosr = wk.tile([128, CHUNK_COLS], F32, tag="cosr")
                    nc.vector.tensor_reduce(cosr[:, :cols],
                                            prod[:, :cols, :],
                                            axis=AX.X, op=OP.add)
                    ee = wk.tile([128, CHUNK_COLS], F32, tag="ee")
                    nc.scalar.activation(ee[:, :cols], cosr[:, :cols], AF.Exp,
                                         scale=(betasb[:] if use_beta
                                                else 1.0))
                    em2 = wk.tile([128, CHUNK_COLS], F32, tag="em2")
                    nc.vector.tensor_tensor(
                        out=em2[:, :cols], in0=ee[:, :cols],
                        in1=Gs[:, :cols, H:H + 1].rearrange(
                            "p c o -> p (c o)"),
                        op=OP.mult)
                    wf = wk.tile([128, CHUNK_COLS * H], BF16, tag="wf")
                    for (t0, T, K, off) in ch["runs"]:
                        wv = wf[:, off * H:(off + T * K) * H].rearrange(
                            "p (t h k) -> p t h k", h=H, k=K)
                        nc.vector.tensor_tensor(
                            out=wv,
                            in0=Gs[:, off:off + T * K, 0:H].rearrange(
                                "p (t k) h -> p t h k", k=K),
                            in1=_bc(em2[:, off:off + T * K].rearrange(
                                "p (t k) -> p t k", k=K).unsqueeze(2),
                                [128, T, H, K]),
                            op=OP.mult)
                        nc.vector.tensor_reduce(
                            h1nm[:, t0:t0 + T, :], wv, axis=AX.X, op=OP.add)
                        nc.vector.tensor_reduce(
                            sgrp[:, t0:t0 + T],
                            ee[:, off:off + T * K].rearrange(
                                "p (t k) -> p t k", k=K),
                            axis=AX.X, op=OP.add)
                # epilogue: pad correction + exact self-loop + normalize
                nc.vector.tensor_tensor(out=sgrp[:], in0=sgrp[:],
                                        in1=negpadsb[:], op=OP.add)
                nc.vector.tensor_tensor(out=sgrp[:], in0=sgrp[:],
                                        in1=selfee[:], op=OP.add)
                rs = one.tile([128, NT], F32, tag="rs")
                nc.vector.reciprocal(rs[:], sgrp[:])
                tmp = one.tile([128, NT, H], F32, tag="tmph")
                nc.vector.tensor_tensor(
                    out=tmp[:], in0=h0nm[:],
                    in1=_bc(selfee[:].unsqueeze(2), [128, NT, H]),
                    op=OP.mult)
                nc.vector.tensor_tensor(out=h1nm[:], in0=h1nm[:], in1=tmp[:],
                                        op=OP.add)
                nc.vector.tensor_tensor(
                    out=h0nm[:], in0=h1nm[:],
                    in1=_bc(rs[:].unsqueeze(2), [128, NT, H]),
                    op=OP.mult)

            normalize_and_share(0, use_beta=False)
            prop(0, use_beta=False)
            normalize_and_share(1, use_beta=True)
            prop(1, use_beta=True)

            # ------------- classifier (rank-1 updates) + log_softmax -------
            lg = st.tile([128, NT, C], F32)
            tmpc = one.tile([128, NT, C], F32, tag="tmpc")
            for h in range(H):
                dst = lg if h == 0 else tmpc
                nc.vector.tensor_tensor(
                    out=dst[:],
                    in0=_bc(h0nm[:, :, h:h + 1], [128, NT, C]),
                    in1=_bc(w2rsb[:, h * C:(h + 1) * C].unsqueeze(1),
                            [128, NT, C]),
                    op=OP.mult)
                if h > 0:
                    nc.vector.tensor_tensor(out=lg[:], in0=lg[:],
                                            in1=tmpc[:], op=OP.add)
            nc.vector.tensor_tensor(
                out=lg[:], in0=lg[:],
                in1=_bc(b2rsb[:].unsqueeze(1), [128, NT, C]), op=OP.add)
            m7 = one.tile([128, NT], F32, tag="m7")
            nc.vector.tensor_reduce(m7[:], lg[:], axis=AX.X, op=OP.max)
            zm = one.tile([128, NT, C], F32, tag="zm")
            nc.vector.tensor_tensor(
                out=zm[:], in0=lg[:],
                in1=_bc(m7[:].unsqueeze(2), [128, NT, C]),
                op=OP.subtract)
            ez = one.tile([128, NT, C], F32, tag="ez")
            nc.scalar.activation(ez[:].rearrange("p t c -> p (t c)"),
                                 zm[:].rearrange("p t c -> p (t c)"), AF.Exp)
            s7 = one.tile([128, NT], F32, tag="s7")
            nc.vector.tensor_reduce(s7[:], ez[:], axis=AX.X, op=OP.add)
            l7 = one.tile([128, NT], F32, tag="l7")
            nc.scalar.activation(l7[:], s7[:], AF.Ln)
            outsb = one.tile([128, NT, C], F32, tag="outsb")
            nc.vector.tensor_tensor(
                out=outsb[:], in0=zm[:],
                in1=_bc(l7[:].unsqueeze(2), [128, NT, C]),
                op=OP.subtract)
            nc.sync.dma_start(
                out=out_d[:].rearrange("(p t) c -> p t c", p=128),
                in_=outsb[:])

    nc.compile()
    return nc


_CACHE = {}


def kernel(x, edge_index, W1, b1, beta, W2, b2):
    x = np.asarray(x, dtype=np.float32)
    edge_index = np.asarray(edge_index)
    in_maps, meta = _host_prep(x, edge_index, np.asarray(W1), np.asarray(b1),
                               np.asarray(beta), np.asarray(W2),
                               np.asarray(b2))
    key = (meta["CTOT"], tuple(meta["Kprof"].tolist()))
    if _CACHE.get("key") != key:
        _CACHE["prog"] = _build_program(meta)
        _CACHE["key"] = key
    nc = _CACHE["prog"]
    res = run_bass_kernel_spmd(nc, in_maps, list(range(NC_CORES)),
                               trace=TRACE[0])
    LAST_EXEC_NS[0] = res.exec_time_ns
    out = np.empty((N, C), dtype=np.float32)
    r = np.arange(L)
    for c in range(NC_CORES):
        oc = res.results[c]["out"].reshape(128, NT, C)
        out[c * L + meta["perms"][c]] = oc[r % 128, r // 128]
    return out



# revision 12
# speedup vs baseline: 1.1185x; 1.1185x over previous
"""AGNN (2-layer propagation) Trainium2 Bass kernel, 8-core SPMD — V2.

Architecture (dst-node-range sharding, 12500 nodes/core, LP=12544):
  - phase A: h0 = relu(x_local @ W1 + b1) via PE (node-major output).
  - per prop: L2-normalize, pack records (hn||norm) 4/256B-window, AllGather
    table to all cores; per-edge phase gathers source windows via gpsimd
    dma_gather into a flat ELL column stream (dst-row-major, degree-sorted
    rank permutation, equal-K tile runs), then computes the segment softmax
    and weighted sums with CONSOLIDATED per-chunk/per-run DVE ops (4D
    broadcast/strided APs) instead of per-tile ops.
  - self-loops are folded analytically (exact, via on-chip self-cos), ELL
    row pads are corrected by a precomputed pad count; softmax is shift-free
    (logits = beta*cos bounded).
  - classifier uses 16 DVE rank-1 updates (no PE transposes), log_softmax.
"""
import sys
import types

sys.path.insert(0, "/opt/trn_rl_repo")

import numpy as np

try:  # optional NTFF profiling hook (enabled by test.py via TRACE)
    import antenv
    if "antenv.axon_hooks" not in sys.modules:
        _hook = [None]
        _m = types.ModuleType("antenv.axon_hooks")
        _m.set_axon_ntff_profile_hook = lambda h: _hook.__setitem__(0, h)
        _m.get_axon_ntff_profile_hook = lambda: _hook[0]
        sys.modules["antenv.axon_hooks"] = _m
        antenv.axon_hooks = _m
        try:
            from trn_agent_boot.trn_boot import _ntff_profile_via_ctypes
            _m.set_axon_ntff_profile_hook(
                _ntff_profile_via_ctypes("/opt/axon/libaxon_pjrt.so"))
        except Exception:
            pass
except Exception:
    pass

import concourse.bass as bass  # noqa: F401
import concourse.mybir as mybir
import concourse.tile as tile
from concourse import bacc
from concourse import library_config
from concourse.bass_utils import run_bass_kernel_spmd
from concourse.masks import make_identity

F32 = mybir.dt.float32
BF16 = mybir.dt.bfloat16
U8 = mybir.dt.uint8
I16 = mybir.dt.int16
AF = mybir.ActivationFunctionType
OP = mybir.AluOpType
AX = mybir.AxisListType

NC_CORES = 8
N = 100000
F_IN = 1433
H = 16
C = 7
EPS = 1e-12

L = N // NC_CORES            # 12500 local nodes per core
LP = 12544                   # padded (98 tiles of 128)
NT = LP // 128               # 98 tiles
KP = 1536                    # padded contraction dim (12 x 128)
NW = NC_CORES * LP // 4      # 25088 real windows
DUMMY_W = NW                 # zero window
NTAB = NW + 4

NIDX_CALL = 1024             # dma_gather idxs per call (ucode max)
CC = NIDX_CALL // 128        # gather-call columns (8)
COL_W = NIDX_CALL // 16      # wrapped idx columns per call (64)
CHUNK_COLS = 96              # max ELL columns per G4 chunk

TRACE = [False]
LAST_EXEC_NS = [None]


def _bc(ap, shape):
    try:
        return ap.broadcast_to(shape)
    except Exception:
        return ap.to_broadcast(shape)


def _plan(Kprof):
    """Chunk the tile sequence into gather chunks (<= CHUNK_COLS columns,
    padded to CC multiples) of equal-K runs."""
    chunks = []
    t = 0
    col0 = 0
    while t < NT:
        tiles = []
        cols = 0
        while t < NT and cols + int(Kprof[t]) <= CHUNK_COLS:
            tiles.append(t)
            cols += int(Kprof[t])
            t += 1
        assert tiles, f"tile K {Kprof[t]} exceeds CHUNK_COLS"
        # equal-K runs within the chunk
        runs = []
        i = 0
        off = 0
        while i < len(tiles):
            K = int(Kprof[tiles[i]])
            j = i
            while j < len(tiles) and int(Kprof[tiles[j]]) == K:
                j += 1
            T = j - i
            if K > 0:
                runs.append((tiles[i], T, K, off))
            off += T * K
            i = j
        cols_pad = -(-cols // CC) * CC
        chunks.append(dict(tiles=tiles, runs=runs, cols=cols,
                           cols_pad=cols_pad, ncalls=cols_pad // CC,
                           col0=col0))
        col0 += cols_pad
    return chunks, col0


def _host_prep(x, edge_index, W1, b1, beta, W2, b2):
    src_all = edge_index[0].astype(np.int64)
    dst_all = edge_index[1].astype(np.int64)
    deg = np.bincount(dst_all, minlength=N)  # real edges only; loops folded

    perms = []
    deg_sorted = np.zeros((NC_CORES, LP), dtype=np.int64)
    for c in range(NC_CORES):
        dl = deg[c * L:(c + 1) * L]
        order = np.argsort(-dl, kind="stable")
        perms.append(order)
        deg_sorted[c, :L] = dl[order]

    Kprof = deg_sorted[:, ::128].max(axis=0).astype(np.int64)  # [NT]
    chunks, CTOT = _plan(Kprof)
    ncalls = CTOT // CC

    # tile -> (chunk col0 + in-chunk offset)
    tile_col = np.zeros(NT, dtype=np.int64)
    for ch in chunks:
        off = 0
        for t in ch["tiles"]:
            tile_col[t] = ch["col0"] + off
            off += int(Kprof[t])

    rank_of = np.empty(N, dtype=np.int64)
    for c in range(NC_CORES):
        rank_of[c * L + perms[c]] = np.arange(L)
    src_rank = rank_of[src_all]
    src_gp = (src_all // L) * LP + (src_rank % 128) * NT + (src_rank // 128)
    dstc = dst_all // L
    dst_rank = rank_of[dst_all]

    idx_streams = np.empty((NC_CORES, 128, ncalls * COL_W), dtype=np.int16)
    msks = np.zeros((NC_CORES, 128, 3 * CTOT), dtype=np.uint8)
    negpad = np.zeros((NC_CORES, 128, NT), dtype=np.float32)

    for c in range(NC_CORES):
        sel = dstc == c
        dr = dst_rank[sel]
        gp = src_gp[sel]
        o = np.argsort(dr, kind="stable")
        dr = dr[o]
        gp = gp[o]
        starts = np.searchsorted(dr, np.arange(LP))
        pos = np.arange(len(dr)) - starts[dr]
        tt = dr // 128
        pp = dr % 128
        col = tile_col[tt] + pos
        iw = np.full((128, CTOT), DUMMY_W, dtype=np.int64)
        sub = np.zeros((128, CTOT), dtype=np.int64)
        iw[pp, col] = gp >> 2
        sub[pp, col] = gp & 3
        real = np.zeros((128, CTOT), dtype=bool)
        real[pp, col] = True
        for j in (1, 2, 3):
            msks[c, :, (j - 1) * CTOT:j * CTOT] = (sub == j) & real
        # chunk-tail pad columns: idx -1 => dma_gather emits no descriptors
        for ch in chunks:
            c0 = ch["col0"]
            iw[:, c0 + ch["cols"]:c0 + ch["cols_pad"]] = -1

        lin = iw.T.reshape(ncalls, NIDX_CALL)  # col-major per call
        wr = lin.reshape(ncalls, COL_W, 16).transpose(0, 2, 1)
        flat = wr.transpose(1, 0, 2).reshape(16, ncalls * COL_W)
        for rep in range(8):
            idx_streams[c, rep * 16:(rep + 1) * 16, :] = flat

        kmat = np.repeat(Kprof[None, :], 128, axis=0)  # [128, NT]
        dmat = deg_sorted[c].reshape(NT, 128).T        # [p, t]
        negpad[c] = -(kmat - dmat).astype(np.float32)

    W1p = np.zeros((KP, H), dtype=np.float32)
    W1p[:F_IN] = W1
    in_maps = []
    for c in range(NC_CORES):
        xt = np.zeros((KP, LP), dtype=np.float32)
        xt[:F_IN, :L] = x[c * L + perms[c]].T
        in_maps.append({
            "xt": xt.astype(np.dtype("bfloat16")),
            "w1": W1p.astype(np.dtype("bfloat16")),
            "b1": b1.reshape(H, 1).astype(np.float32),
            "w2r": np.tile(W2.reshape(1, H * C), (128, 1)).astype(np.float32),
            "b2r": np.tile(b2.reshape(1, C), (128, 1)).astype(np.float32),
            "betar": np.full((128, 1), float(beta[0]), dtype=np.float32),
            "idxs": idx_streams[c],
            "msk": msks[c],
            "negpad": negpad[c],
        })
    call_valid = np.full(ncalls, NIDX_CALL, dtype=np.int64)
    for ch in chunks:
        cb = ch["col0"] // CC
        for i in range(ch["ncalls"]):
            lo = i * CC
            hi = min(ch["cols"] - lo, CC)
            call_valid[cb + i] = max(hi, 0) * 128
    meta = dict(chunks=chunks, CTOT=CTOT, ncalls=ncalls, Kprof=Kprof,
                perms=perms, call_valid=call_valid)
    return in_maps, meta


def _build_program(meta):
    chunks = meta["chunks"]
    CTOT = meta["CTOT"]
    ncalls_total = meta["ncalls"]
    call_valid = meta["call_valid"]

    nc = bacc.Bacc("TRN2", target_bir_lowering=False, debug=False,
                   num_devices=NC_CORES, num_swdge_queues=4)

    xt_d = nc.dram_tensor("xt", [KP, LP], BF16, kind="ExternalInput")
    w1_d = nc.dram_tensor("w1", [KP, H], BF16, kind="ExternalInput")
    b1_d = nc.dram_tensor("b1", [H, 1], F32, kind="ExternalInput")
    w2r_d = nc.dram_tensor("w2r", [128, H * C], F32, kind="ExternalInput")
    b2r_d = nc.dram_tensor("b2r", [128, C], F32, kind="ExternalInput")
    betar_d = nc.dram_tensor("betar", [128, 1], F32, kind="ExternalInput")
    idxs_d = nc.dram_tensor("idxs", [128, ncalls_total * COL_W], I16,
                            kind="ExternalInput")
    msk_d = nc.dram_tensor("msk", [128, 3 * CTOT], U8, kind="ExternalInput")
    negpad_d = nc.dram_tensor("negpad", [128, NT], F32, kind="ExternalInput")
    out_d = nc.dram_tensor("out", [LP, C], F32, kind="ExternalOutput")

    rec_d = [nc.dram_tensor(f"rec{i}", [128, NT, 32], BF16) for i in range(2)]
    tab_d = [nc.dram_tensor(f"tab{i}", [NTAB, 128], BF16, addr_space="Shared")
             for i in range(2)]

    qctr = [0]

    def nextq():
        q = qctr[0] % 4
        qctr[0] += 1
        return q

    with tile.TileContext(nc) as tc:
        with tc.tile_pool(name="const", bufs=1) as cst, \
             tc.tile_pool(name="state", bufs=1) as st, \
             tc.tile_pool(name="work", bufs=3) as wk, \
             tc.tile_pool(name="once", bufs=1) as one, \
             tc.tile_pool(name="gath", bufs=3) as gp:

            nc.gpsimd.load_library(library_config.mlp)

            w1sb = cst.tile([128, 12, H], BF16)
            for kt in range(12):
                nc.sync.dma_start(out=w1sb[:, kt, :],
                                  in_=w1_d[kt * 128:(kt + 1) * 128, :])
            b1sb = cst.tile([H, 1], F32)
            nc.sync.dma_start(out=b1sb[:], in_=b1_d[:])
            w2rsb = cst.tile([128, H * C], F32)
            nc.sync.dma_start(out=w2rsb[:], in_=w2r_d[:])
            b2rsb = cst.tile([128, C], F32)
            nc.sync.dma_start(out=b2rsb[:], in_=b2r_d[:])
            betasb = cst.tile([128, 1], F32)
            nc.sync.dma_start(out=betasb[:], in_=betar_d[:])
            epssb = cst.tile([128, 1], F32)
            nc.vector.memset(epssb[:], EPS)
            msksb = cst.tile([128, 3 * CTOT], U8)
            nc.sync.dma_start(out=msksb[:], in_=msk_d[:])
            negpadsb = cst.tile([128, NT], F32)
            nc.sync.dma_start(out=negpadsb[:], in_=negpad_d[:])
            ident128 = cst.tile([128, 128], F32)
            make_identity(nc, ident128[:])
            zer = cst.tile([1, 128], BF16)
            nc.vector.memset(zer[:], 0)
            for i in range(2):
                nc.sync.dma_start(out=tab_d[i][NW:NW + 1, :], in_=zer[:])

            # ------------- phase A: h0 = relu(x W1 + b1), node-major -------
            h0nm = st.tile([128, NT, H], F32)
            with tc.tile_pool(name="psA", bufs=2, space="PSUM") as psA, \
                 tc.tile_pool(name="psTa", bufs=4, space="PSUM") as psTa, \
                 tc.tile_pool(name="wkA", bufs=2) as wkA:
                CH = 512
                for coff in range(0, LP, CH):
                    csz = min(CH, LP - coff)
                    ps = psA.tile([H, CH], F32, tag="psa")
                    xtile = wkA.tile([128, 12, CH], BF16, tag="xt")
                    nc.sync.dma_start(
                        out=xtile[:, :, :csz],
                        in_=xt_d[:, coff:coff + csz].rearrange(
                            "(k p) c -> p k c", p=128))
                    for kt in range(12):
                        for m in range(0, csz, 512):
                            mw = min(512, csz - m)
                            nc.tensor.matmul(ps[:, m:m + mw],
                                             lhsT=w1sb[:, kt, :],
                                             rhs=xtile[:, kt, m:m + mw],
                                             start=(kt == 0), stop=(kt == 11))
                    hfm = wkA.tile([H, CH], F32, tag="hfm")
                    nc.scalar.activation(hfm[:, :csz], ps[:, :csz], AF.Relu,
                                         bias=b1sb[:])
                    for i in range(csz // 128):
                        tg = (coff + i * 128) // 128
                        pt = psTa.tile([128, H], F32, tag="pst")
                        nc.tensor.transpose(
                            out=pt[:], in_=hfm[:, i * 128:(i + 1) * 128],
                            identity=ident128[:H, :H])
                        nc.scalar.activation(h0nm[:, tg, :], pt[:], AF.Copy)

            hnbf = st.tile([128, NT, H], BF16)
            h1nm = st.tile([128, NT, H], F32)
            sgrp = st.tile([128, NT], F32)
            selfee = st.tile([128, NT], F32)

            def normalize_and_share(phase, use_beta):
                hh = one.tile([128, NT * H], F32, tag="hh")
                nc.scalar.activation(
                    hh[:], h0nm[:].rearrange("p t h -> p (t h)"), AF.Square)
                ss = one.tile([128, NT], F32, tag="ss")
                nc.vector.tensor_reduce(
                    ss[:], hh[:].rearrange("p (t h) -> p t h", h=H),
                    axis=AX.X, op=OP.add)
                sq = one.tile([128, NT], F32, tag="sq")
                nc.scalar.activation(sq[:], ss[:], AF.Sqrt, bias=epssb[:])
                rr = one.tile([128, NT], F32, tag="rr")
                nc.vector.reciprocal(rr[:], sq[:])
                # self-cos = ss/(ss+eps); exact self-loop term
                sc = one.tile([128, NT], F32, tag="sc")
                nc.vector.tensor_tensor(out=sc[:], in0=ss[:], in1=rr[:],
                                        op=OP.mult)
                nc.vector.tensor_tensor(out=sc[:], in0=sc[:], in1=rr[:],
                                        op=OP.mult)
                nc.scalar.activation(selfee[:], sc[:], AF.Exp,
                                     scale=(betasb[:] if use_beta else 1.0))
                nc.vector.tensor_tensor(
                    out=hnbf[:], in0=h0nm[:],
                    in1=_bc(rr[:].unsqueeze(2), [128, NT, H]),
                    op=OP.mult)
                rec = one.tile([128, NT, 32], BF16, tag="rec")
                nc.vector.tensor_copy(out=rec[:, :, 0:H], in_=hnbf[:])
                nc.vector.tensor_copy(out=rec[:, :, H:H + 1],
                                      in_=sq[:].unsqueeze(2))
                nc.sync.dma_start(out=rec_d[phase][:], in_=rec[:])
                nc.gpsimd.collective_compute(
                    "AllGather", OP.bypass,
                    replica_groups=[list(range(NC_CORES))],
                    ins=[rec_d[phase][:]],
                    outs=[tab_d[phase][0:NW, :]],
                )

            def prop(phase, use_beta):
                nc.vector.memset(h1nm[:], 0)
                nc.vector.memset(sgrp[:], 0)
                for ch in chunks:
                    cols = ch["cols"]
                    ncalls = ch["ncalls"]
                    cb = ch["col0"] // CC
                    idxsb = wk.tile([128, (CHUNK_COLS // CC) * COL_W], I16,
                                    tag="idx")
                    nc.sync.dma_start(
                        out=idxsb[:, :ncalls * COL_W],
                        in_=idxs_d[:, cb * COL_W:(cb + ncalls) * COL_W])
                    G4 = gp.tile([128, CHUNK_COLS, 128], BF16, tag="g4")
                    for i in range(ncalls):
                        vv = int(call_valid[cb + i])
                        if vv == 0:
                            continue
                        nc.gpsimd.dma_gather(
                            out_ap=G4[:, i * CC:(i + 1) * CC, :],
                            in_ap=tab_d[phase][:],
                            idxs_ap=idxsb[:, i * COL_W:(i + 1) * COL_W],
                            num_idxs=NIDX_CALL,
                            num_idxs_reg=vv,
                            elem_size=128,
                            queue_num=nextq(),
                        )
                    Gs = wk.tile([128, CHUNK_COLS, 18], BF16, tag="gs")
                    nc.scalar.activation(Gs[:, :cols, :], G4[:, :cols, 0:18],
                                         AF.Copy)
                    mc0 = ch["col0"]
                    for j in (1, 2, 3):
                        mj = msksb[:, (j - 1) * CTOT + mc0:
                                   (j - 1) * CTOT + mc0 + cols]
                        nc.vector.copy_predicated(
                            out=Gs[:, :cols, :],
                            mask=_bc(mj.unsqueeze(2), [128, cols, 18]),
                            data=G4[:, :cols, 32 * j:32 * j + 18])
                    prod = wk.tile([128, CHUNK_COLS, H], BF16, tag="prod")
                    for (t0, T, K, off) in ch["runs"]:
                        nc.vector.tensor_tensor(
                            out=prod[:, off:off + T * K, :].rearrange(
                                "p (t k) h -> p t k h", k=K),
                            in0=Gs[:, off:off + T * K, 0:H].rearrange(
                                "p (t k) h -> p t k h", k=K),
                            in1=_bc(hnbf[:, t0:t0 + T, :].unsqueeze(2),
                                    [128, T, K, H]),
                            op=OP.mult)
                    cosr = wk.tile([128, CHUNK_COLS], F32, tag="cosr")
                    nc.vector.tensor_reduce(cosr[:, :cols],
                                            prod[:, :cols, :],
                                            axis=AX.X, op=OP.add)
                    ee = wk.tile([128, CHUNK_COLS], F32, tag="ee")
                    nc.scalar.activation(ee[:, :cols], cosr[:, :cols], AF.Exp,
                                         scale=(betasb[:] if use_beta
                                                else 1.0))
                    em2 = wk.tile([128, CHUNK_COLS], F32, tag="em2")
                    nc.vector.tensor_tensor(
                        out=em2[:, :cols], in0=ee[:, :cols],
                        in1=Gs[:, :cols, H:H + 1].rearrange(
                            "p c o -> p (c o)"),
                        op=OP.mult)
                    wf = wk.tile([128, CHUNK_COLS * H], BF16, tag="wf")
                    for (t0, T, K, off) in ch["runs"]:
                        wv = wf[:, off * H:(off + T * K) * H].rearrange(
                            "p (t h k) -> p t h k", h=H, k=K)
                        nc.vector.tensor_tensor(
                            out=wv,
                            in0=Gs[:, off:off + T * K, 0:H].rearrange(
                                "p (t k) h -> p t h k", k=K),
                            in1=_bc(em2[:, off:off + T * K].rearrange(
                                "p (t k) -> p t k", k=K).unsqueeze(2),
                                [128, T, H, K]),
                            op=OP.mult)
                        nc.vector.tensor_reduce(
                            h1nm[:, t0:t0 + T, :], wv, axis=AX.X, op=OP.add)
                        nc.vector.tensor_reduce(
                            sgrp[:, t0:t0 + T],
                            ee[:, off:off + T * K].rearrange(
                                "p (t k) -> p t k", k=K),
                            axis=AX.X, op=OP.add)
                # epilogue: pad correction + exact self-loop + normalize
                nc.vector.tensor_tensor(out=sgrp[:], in0=sgrp[:],
                                        in1=negpadsb[:], op=OP.add)
                nc.vector.tensor_tensor(out=sgrp[:], in0=sgrp[:],
                                        in1=selfee[:], op=OP.add)
                rs = one.tile([128, NT], F32, tag="rs")
                nc.vector.reciprocal(rs[:], sgrp[:])
                tmp = one.tile([128, NT, H], F32, tag="tmph")
                nc.vector.tensor_tensor(
                    out=tmp[:], in0=h0nm[:],
                    in1=_bc(selfee[:].unsqueeze(2), [128, NT, H]),
                    op=OP.mult)
                nc.vector.tensor_tensor(out=h1nm[:], in0=h1nm[:], in1=tmp[:],
                                        op=OP.add)
                nc.vector.tensor_tensor(
                    out=h0nm[:], in0=h1nm[:],
                    in1=_bc(rs[:].unsqueeze(2), [128, NT, H]),
                    op=OP.mult)

            normalize_and_share(0, use_beta=False)
            prop(0, use_beta=False)
            normalize_and_share(1, use_beta=True)
            prop(1, use_beta=True)

            # ------------- classifier (rank-1 updates) + log_softmax -------
            lg = st.tile([128, NT, C], F32)
            tmpc = one.tile([128, NT, C], F32, tag="tmpc")
            for h in range(H):
                dst = lg if h == 0 else tmpc
                nc.vector.tensor_tensor(
                    out=dst[:],
                    in0=_bc(h0nm[:, :, h:h + 1], [128, NT, C]),
                    in1=_bc(w2rsb[:, h * C:(h + 1) * C].unsqueeze(1),
                            [128, NT, C]),
                    op=OP.mult)
                if h > 0:
                    nc.vector.tensor_tensor(out=lg[:], in0=lg[:],
                                            in1=tmpc[:], op=OP.add)
            nc.vector.tensor_tensor(
                out=lg[:], in0=lg[:],
                in1=_bc(b2rsb[:].unsqueeze(1), [128, NT, C]), op=OP.add)
            m7 = one.tile([128, NT], F32, tag="m7")
            nc.vector.tensor_reduce(m7[:], lg[:], axis=AX.X, op=OP.max)
            zm = one.tile([128, NT, C], F32, tag="zm")
            nc.vector.tensor_tensor(
                out=zm[:], in0=lg[:],
                in1=_bc(m7[:].unsqueeze(2), [128, NT, C]),
                op=OP.subtract)
            ez = one.tile([128, NT, C], F32, tag="ez")
            nc.scalar.activation(ez[:].rearrange("p t c -> p (t c)"),
                                 zm[:].rearrange("p t c -> p (t c)"), AF.Exp)
            s7 = one.tile([128, NT], F32, tag="s7")
            nc.vector.tensor_reduce(s7[:], ez[:], axis=AX.X, op=OP.add)
            l7 = one.tile([128, NT], F32, tag="l7")
            nc.scalar.activation(l7[:], s7[:], AF.Ln)
            outsb = one.tile([128, NT, C], F32, tag="outsb")
            nc.vector.tensor_tensor(
                out=outsb[:], in0=zm[:],
                in1=_bc(l7[:].unsqueeze(2), [128, NT, C]),
                op=OP.subtract)
            nc.sync.dma_start(
                out=out_d[:].rearrange("(p t) c -> p t c", p=128),
                in_=outsb[:])

    nc.compile()
    return nc


_CACHE = {}


def kernel(x, edge_index, W1, b1, beta, W2, b2):
    x = np.asarray(x, dtype=np.float32)
    edge_index = np.asarray(edge_index)
    in_maps, meta = _host_prep(x, edge_index, np.asarray(W1), np.asarray(b1),
                               np.asarray(beta), np.asarray(W2),
                               np.asarray(b2))
    key = (meta["CTOT"], tuple(meta["Kprof"].tolist()))
    if _CACHE.get("key") != key:
        _CACHE["prog"] = _build_program(meta)
        _CACHE["key"] = key
    nc = _CACHE["prog"]
    res = run_bass_kernel_spmd(nc, in_maps, list(range(NC_CORES)),
                               trace=TRACE[0])
    LAST_EXEC_NS[0] = res.exec_time_ns
    out = np.empty((N, C), dtype=np.float32)
    r = np.arange(L)
    for c in range(NC_CORES):
        oc = res.results[c]["out"].reshape(128, NT, C)
        out[c * L + meta["perms"][c]] = oc[r % 128, r // 128]
    return out

